# revision 46
# baseline (speedup 1.0000x reference)
"""Expert-parallel BaseLayer MoE kernel for 8 TRN2 NeuronCores (v7).

Host does everything cheap and precision-critical in exact f32: routing
(argmax affinities), layernorm stats + normalization, the sigmoid gate, the
residual and both biases' final application.  Each core owns one expert at
capacity C = ceil(T/E) = 256 (capacity factor 1.0); the few overflow tokens
of over-subscribed experts (~1.7% of tokens) are computed exactly on the
host, as a real All2All-capacity MoE would drop/overflow them.  The device
computes ONLY the FLOP-heavy part: z = relu(xq @ w1q + SW1*b1),
delta = z @ w2q, returned as bf16 (scaled by SW1*SW2; host divides).

Performance structure (measured on TRN2):
 - mm1 all-fp8 DoubleRow, mm2 NF8=14 fp8 + 18 bf16 k-tiles: the measured
   error/time frontier point at rel-err 1.95e-2 vs the 2e-2 gate.
 - every DR matmul is a single pass per k-pair with moving dim 2*cw=512
   (1 token/cycle at K=256 = the 157 TF/s fp8 peak; LDWEIGHTS shadow-loads
   under the previous matmul).
 - all input DMAs ride the sync HWDGE ring in strict consumption order;
   each dma_start costs ~0.65us issue + ~0.9us completion receipt, so the
   first piece packs [x | w1_j0] into one fp8 blob for the earliest
   possible matmul start, later pieces grow geometrically, and the fp8 w2
   pieces interleave into mm1's tail slack.
 - activations: fp8-z tiles on Scalar ACTIVATE (relu+bias+cast), bf16-z
   tiles on Vector tensor_scalar(add,max) - neither engine backpressures.
 - a burst of full-width dummy matmuls at kernel start keeps the PE at
   ~100% duty through the DMA dead time so the HAM clock-gate (default
   4/8 = 1.2 GHz) opens to 8/8 right as real matmuls begin.
 - last output tile is computed as two column-half PSUM chains so the
   first half's cast+DMA hides under the second half's matmuls.
"""

import functools
import sys

import numpy as np

for _p in ("/opt/trn_rl_repo", "/opt/pypackages"):
    if _p not in sys.path:
        sys.path.append(_p)

import ml_dtypes  # noqa: E402

import concourse.bass as bass  # noqa: E402
import concourse.mybir as mybir  # noqa: E402
import concourse.tile as tile  # noqa: E402
from concourse import bacc  # noqa: E402
from concourse import bass_utils  # noqa: E402


def _ensure_axon_hooks():
    """bass_utils' trace path imports antenv.axon_hooks, which some agent
    images lack; synthesize it (with the real ctypes NTFF hook when
    available) so tracing degrades gracefully instead of crashing."""
    try:
        import antenv.axon_hooks  # noqa: F401
        return
    except ImportError:
        pass
    import types

    import antenv

    hooks = types.ModuleType("antenv.axon_hooks")
    hooks._hook = None
    hooks.set_axon_ntff_profile_hook = lambda h: setattr(hooks, "_hook", h)
    hooks.get_axon_ntff_profile_hook = lambda: hooks._hook
    sys.modules["antenv.axon_hooks"] = hooks
    antenv.axon_hooks = hooks
    try:
        from trn_agent_boot.trn_boot import _ntff_profile_via_ctypes

        hooks._hook = _ntff_profile_via_ctypes("/opt/axon/libaxon_pjrt.so")
    except Exception:
        pass


_ensure_axon_hooks()

E = 8
D = 1024
F = 4096
EPS = 1e-5
KD = D // 128    # 8 k-tiles over d
KF = F // 128    # 32 k-tiles over f
CAP = 256        # per-expert device capacity (= mean load); overflow on host
SW1 = 32.0       # host-side w1 pre-scale (fp8 subnormal avoidance); z stored as SW1*z
SW2 = 8.0        # host-side w2 pre-scale (fp8 subnormal avoidance)
NF8 = 16         # k2-tiles of matmul-2 run in fp8 DoubleRow (of KF=32); rest bf16
MBF = KF - NF8   # bf16 k2-tiles in matmul-2
WARM = 16        # dummy PE-warmup matmuls (HAM clock-gate ramp); full-width
                 # moving dim so PE duty stays ~100% (HAM ignores low duty).
                 # Sized to bridge until P0's completion semaphore (data
                 # lands ~9.3 but the straggler-engine receipt adds ~2us).
NFILL = 12       # filler matmuls woven between the first real j's: they
                 # absorb per-piece DMA-completion jitter so the PE never
                 # idles long enough for a HAM re-throttle.

F32 = mybir.dt.float32
BF16 = mybir.dt.bfloat16
F8 = mybir.dt.float8e4
AF = mybir.ActivationFunctionType
DR = mybir.MatmulPerfMode.DoubleRow
ALU = mybir.AluOpType

NP_BF16 = ml_dtypes.bfloat16
NP_F8 = ml_dtypes.float8_e4m3

# w1 DMA piece sizes in j's for j >= 1 (j0 rides the packed first piece).
# Each dma_start costs ~0.65us of issue time and pieces land in FIFO order;
# 4-j pieces keep the supply cushion growing against mm1's consumption.
W1_PIECES = [2, 3, 4, 4, 4, 4, 4, 4, 2]
# after which w1 piece index (0-based) to interleave each per-i w28 piece
W28_AFTER = [5, 6, 7, 8, 8, 8, 8, 8]

# mm1 j emission order: interleave fp8-z j's (scalar ACT) with bf16-z j's
# (vector tensor_scalar) so the two activation engines alternate and neither
# backpressures the matmul stream.  Starts at j0 (pre-staged in P0); the w1
# DMA stream is permuted to this order on the host.  The last fp8 j lands a
# few slots before the end so z8 is complete when mm2's DR passes start.
J_ORDER = []
for _k in range(NF8):
    J_ORDER.append(_k)
    J_ORDER.append(NF8 + _k)
J_ORDER += list(range(2 * NF8, KF))
assert sorted(J_ORDER) == list(range(KF))


@functools.lru_cache(maxsize=4)
def _build(c_total, cw):
    nc = bacc.Bacc("TRN2", target_bir_lowering=False, debug=False, num_devices=E)

    # p0 packs [x (KD*c_total B/part) | w1 j0 (KD*128 B/part)] in fp8
    P0X = KD * c_total
    P0W = KD * 128
    p0d = nc.declare_dram_parameter("p0", [128, P0X + P0W], F8, isOutput=False)
    w1d = nc.declare_dram_parameter("w1q", [128, KF - 1, KD, 128], F8, isOutput=False)
    w28d = nc.declare_dram_parameter("w28", [128, KD, NF8, 128], F8, isOutput=False)
    w2bd = nc.declare_dram_parameter("w2b", [128, KD, MBF, 128], BF16, isOutput=False)
    b1d = nc.declare_dram_parameter("b1s", [128, KF], F32, isOutput=False)
    outd = nc.declare_dram_parameter("out", [128, KD, cw], BF16, isOutput=True)

    with tile.TileContext(nc) as tc:
        with (
            tc.tile_pool(name="const", bufs=1) as constp,
            tc.tile_pool(name="xp", bufs=1) as xp,
            tc.tile_pool(name="w1p", bufs=1) as w1p,
            tc.tile_pool(name="w2p", bufs=1) as w2p,
            tc.tile_pool(name="zp", bufs=1) as zp,
            tc.tile_pool(name="outp", bufs=3) as outp,
            tc.tile_pool(name="ps_w", bufs=1, space=bass.MemorySpace.PSUM) as psw,
            tc.tile_pool(name="ps_z", bufs=3, space=bass.MemorySpace.PSUM) as psz,
            tc.tile_pool(name="ps_y", bufs=2, space=bass.MemorySpace.PSUM) as psy,
            tc.tile_pool(name="ps_t", bufs=1, space=bass.MemorySpace.PSUM) as pst,
        ):
            # --- PE warm-up: HAM un-throttles after a full ~3.4us window of
            # near-100%-duty PE activity.  Run dummy DR matmuls shaped like
            # the real mm1 ones (moving 2*cw) during the DMA wait.
            warm = constp.tile([128, 2, c_total], F8, tag="warm")
            nc.gpsimd.memset(warm[:], 0)
            pw = psw.tile([128, cw], F32, tag="pw")
            for _ in range(WARM):
                nc.tensor.matmul(
                    pw[:], warm[:, :, :128], warm[:, :, :cw],
                    start=True, stop=True, perf_mode=DR,
                )

            # --- all inputs ride the sync (SP) HWDGE ring in strict
            # consumption order (FIFO per engine -> landing order).
            p0_sb = xp.tile([128, P0X + P0W], F8, tag="p0")
            nc.sync.dma_start(out=p0_sb[:], in_=p0d[:])

            def x_pair(kp, sl):
                # rhs AP [2, cw] for x k-pair kp out of the packed p0 tile
                ap = p0_sb[:, bass.ds(2 * kp * c_total, 2 * c_total)]
                return ap.rearrange("p (two c) -> p two c", two=2)[:, :, sl]

            def w1j0_pair(kp):
                # lhsT AP [2, 128] for w1 j0 k-pair kp out of packed p0
                ap = p0_sb[:, bass.ds(P0X + 2 * kp * 128, 256)]
                return ap.rearrange("p (two c) -> p two c", two=2)

            # w1 stream is laid out in J_ORDER[1:] on the host side
            w1_sbs = [None] * KF
            w28_map = {}
            pos = 0
            for pc, nj in enumerate(W1_PIECES):
                t = w1p.tile([128, nj, KD, 128], F8, tag=f"w1_{pc}", name=f"w1_{pc}")
                nc.sync.dma_start(out=t[:], in_=w1d[:, pos : pos + nj])
                for jj in range(nj):
                    w1_sbs[J_ORDER[1 + pos + jj]] = t[:, jj]
                pos += nj
                for i8 in [k for k, a in enumerate(W28_AFTER) if a == pc]:
                    t8 = w2p.tile(
                        [128, NF8, 128], F8, tag=f"w28_{i8}", name=f"w28_{i8}"
                    )
                    nc.sync.dma_start(out=t8[:], in_=w28d[:, i8])
                    w28_map[i8] = t8
            w28_sbs = [w28_map[i] for i in range(KD)]
            w2b_sbs = []
            for i in range(KD):
                tb = w2p.tile([128, MBF, 128], BF16, tag=f"w2b_{i}", name=f"w2b_{i}")
                nc.sync.dma_start(out=tb[:], in_=w2bd[:, i])
                w2b_sbs.append(tb)

            # b1 (tiny, needed only at the first activation) on scalar ring
            b1_sb = constp.tile([128, KF], F32, tag="b1")
            nc.scalar.dma_start(out=b1_sb[:], in_=b1d[:])

            # z tiles: f-tiles < NF8 stored fp8 (consumed by DR), rest bf16
            z8_sb = zp.tile([128, NF8, c_total], F8, tag="z8", name="z8")
            zb_sb = zp.tile([128, MBF, c_total], BF16, tag="zb", name="zb")

            wsl = bass.ds(0, cw)

            # ---- matmul 1 (all fp8 DR, moving=2*cw): z = relu(xq@w1q + SW1*b1)
            for jdx, j in enumerate(J_ORDER):
                if 1 <= jdx <= NFILL:
                    nc.tensor.matmul(
                        pw[:], warm[:, :, :128], warm[:, :, :cw],
                        start=True, stop=True, perf_mode=DR,
                    )
                pz = psz.tile([128, cw], F32, tag="pz")
                for kp in range(KD // 2):
                    lhsT = w1j0_pair(kp) if j == 0 else w1_sbs[j][:, 2 * kp : 2 * kp + 2, :]
                    nc.tensor.matmul(
                        pz[:], lhsT, x_pair(kp, wsl),
                        start=(kp == 0), stop=(kp == KD // 2 - 1),
                        perf_mode=DR,
                    )
                if j < NF8:
                    # fp8 z: Scalar ACT does relu+bias+cast in one op
                    nc.scalar.activation(
                        z8_sb[:, j, wsl], pz[:], AF.Relu,
                        bias=b1_sb[:, j : j + 1], scale=1.0,
                    )
                else:
                    # bf16 z: DVE does max(psum + b1, 0) with cast
                    nc.vector.tensor_scalar(
                        zb_sb[:, j - NF8, wsl], pz[:],
                        b1_sb[:, j : j + 1], 0.0,
                        ALU.add, ALU.max,
                    )

            # ---- matmul 2: delta = z8 @ w28 + zb @ w2b (one PSUM chain/i) ----
            for i in range(KD):
                if i < KD - 1:
                    py = psy.tile([128, cw], F32, tag="py")
                    for p in range(NF8 // 2):
                        nc.tensor.matmul(
                            py[:], w28_sbs[i][:, 2 * p : 2 * p + 2, :],
                            z8_sb[:, 2 * p : 2 * p + 2, wsl],
                            start=(p == 0), stop=False,
                            perf_mode=DR,
                            skip_group_check=(p > 0),
                        )
                    for k in range(MBF):
                        nc.tensor.matmul(
                            py[:], w2b_sbs[i][:, k, :], zb_sb[:, k, wsl],
                            start=False, stop=(k == MBF - 1),
                            skip_group_check=True,
                        )
                    o = outp.tile([128, cw], BF16, tag="o")
                    nc.vector.tensor_copy(o[:], py[:])
                    nc.scalar.dma_start(out=outd[:, i], in_=o[:])
                else:
                    # last i: two independent column-half chains so the
                    # first half's cast+DMA hides under the second half's
                    # matmuls (shorter serial tail)
                    h = (cw // 2 + 7) // 8 * 8
                    o = outp.tile([128, cw], BF16, tag="o")
                    for ci, sl in enumerate((bass.ds(0, h), bass.ds(h, cw - h))):
                        pt = pst.tile(
                            [128, cw - h if ci else h], F32,
                            tag=f"pt{ci}", name=f"pt{ci}",
                        )
                        for p in range(NF8 // 2):
                            nc.tensor.matmul(
                                pt[:], w28_sbs[i][:, 2 * p : 2 * p + 2, :],
                                z8_sb[:, 2 * p : 2 * p + 2, sl],
                                start=(p == 0), stop=False,
                                perf_mode=DR,
                                skip_group_check=(p > 0),
                            )
                        for k in range(MBF):
                            nc.tensor.matmul(
                                pt[:], w2b_sbs[i][:, k, :], zb_sb[:, k, sl],
                                start=False, stop=(k == MBF - 1),
                                skip_group_check=True,
                            )
                        nc.vector.tensor_copy(o[:, sl], pt[:])
                        nc.scalar.dma_start(out=outd[:, i, sl], in_=o[:, sl])

    nc.compile()
    return nc


def kernel(x, centroids, w1, b1, w2, b2, gamma, beta):
    x = np.ascontiguousarray(np.asarray(x, dtype=np.float32))
    centroids = np.asarray(centroids, dtype=np.float32)
    w1 = np.asarray(w1, dtype=np.float32)
    b1 = np.asarray(b1, dtype=np.float32)
    w2 = np.asarray(w2, dtype=np.float32)
    b2 = np.asarray(b2, dtype=np.float32)
    gamma = np.asarray(gamma, dtype=np.float32)
    beta = np.asarray(beta, dtype=np.float32)

    orig_shape = x.shape
    feats = x.reshape(-1, D)

    # routing + stats + gate — exact f32, same math as the reference
    aff = feats @ centroids.T
    eid = np.argmax(aff, axis=1)
    mu = feats.mean(axis=-1, keepdims=True)
    var = feats.var(axis=-1, keepdims=True)
    xhat = (feats - mu) / np.sqrt(var + EPS)

    idxs_all = [np.nonzero(eid == e)[0] for e in range(E)]
    idxs = [ix[:CAP] for ix in idxs_all]        # device tokens (capacity)
    over = [ix[CAP:] for ix in idxs_all]        # host-handled overflow
    counts = [len(ix) for ix in idxs]
    cw = max(64, ((max(counts) + 7) // 8) * 8)
    c_total = ((cw + 31) // 32) * 32

    nc = _build(c_total, cw)

    in_maps = []
    for e in range(E):
        n_e = counts[e]
        xt = np.zeros((128, KD, c_total), dtype=np.float32)
        if n_e:
            xt[:, :, :n_e] = xhat[idxs[e]].T.reshape(KD, 128, n_e).transpose(1, 0, 2)
        w1e = gamma[e][:, None] * w1[e]                      # [D, F]
        b1e = b1[e] + beta[e] @ w1[e]                        # [F]
        w1q = np.ascontiguousarray(
            (w1e * SW1).reshape(KD, 128, KF, 128).transpose(1, 2, 0, 3)
        ).astype(NP_F8)                                      # [128,KF,KD,128]
        p0 = np.concatenate(
            [
                xt.astype(NP_F8).reshape(128, KD * c_total),
                w1q[:, 0].reshape(128, KD * 128),
            ],
            axis=1,
        )
        w2r = (w2[e] * SW2).reshape(KF, 128, KD, 128).transpose(1, 2, 0, 3)
        w28 = np.ascontiguousarray(w2r[:, :, :NF8]).astype(NP_F8)
        w2b = np.ascontiguousarray(w2r[:, :, NF8:]).astype(NP_BF16)
        b1s = np.ascontiguousarray((b1e * SW1).reshape(KF, 128).T)  # [128, KF]
        in_maps.append(
            dict(p0=p0, w1q=np.ascontiguousarray(w1q[:, J_ORDER[1:]]), w28=w28,
                 w2b=w2b, b1s=b1s)
        )

    res = bass_utils.run_bass_kernel_spmd(nc, in_maps, core_ids=list(range(E)))
    kernel._last_res = res

    out = feats.copy()
    inv_scale = 1.0 / (SW1 * SW2)
    for e in range(E):
        n_e = counts[e]
        if n_e:
            d8 = np.asarray(res.results[e]["out"]).astype(np.float32)  # [128,KD,cw]
            delta = d8.transpose(1, 0, 2).reshape(D, cw)[:, :n_e].T * inv_scale
            al = 1.0 / (1.0 + np.exp(-aff[idxs[e], e]))[:, None]
            out[idxs[e]] = feats[idxs[e]] + al * (delta + b2[e])
        if len(over[e]):
            # overflow tokens: exact f32 on host (same math as reference)
            ix = over[e]
            h = xhat[ix] * gamma[e] + beta[e]
            y = feats[ix] + np.maximum(h @ w1[e] + b1[e], 0.0) @ w2[e] + b2[e]
            al = 1.0 / (1.0 + np.exp(-aff[ix, e]))[:, None]
            out[ix] = al * y + (1.0 - al) * feats[ix]
    return out.reshape(orig_shape)
